# revision 18
# baseline (speedup 1.0000x reference)
"""Trainium2 Bass kernel for nn_Attention_30666066493686.

Region-attention over N=36 regions:
  hidden = tanh(region @ Wr + frame @ Wf + b_att)          [T,N,B,A]
  att    = hidden . W_full  (+ b_full, dropped: softmax-shift invariant)
  alpha  = softmax_n(where(mask, -1e9, att))
  out    = sum_n alpha * region                            [T,B,D]

Sharding: data-parallel over T across 8 NeuronCores (4 timesteps each);
params replicated; no collectives.

Host-side prep (inside kernel(), before dispatch): region ships in BOTH
layouts -- natural bf16 [rows, D] for the alpha-weighted sum (phase 2)
and transposed fp8e4m3 [D, rows] for the Wr contraction (phase 1; the
quantization noise is attenuated through tanh+softmax, host-emulated
rel err 5.7e-3 vs the 2e-2 budget) -- so the device does zero on-chip
transposes of region. HBM bytes per timestep: 1.18 MB (fp8) + 2.36 MB
(bf16) = 3.54 MB, 25% under one fp32 copy. frame ships pre-transposed,
mask ships as a pre-chunked f32 keep-mask, all small consts ship packed
in two blobs (one DMA each) issued ahead of the region loads on the SP
HWDGE ring; keep/out traffic rides the ACT ring.

Per-core, per-timestep dataflow (rows = (n,b) flattened = 2304):
  ph[A=128, rows] = fproj-bias-MM + sum_J Wr_J^T @ regionT_J   (PSUM)
  th = tanh(ph + b_att)  (ACT per-partition bias)   -> SBUF bf16
  att[rows] column-ized (lhsT=th chunk, rhs=W_full) -> [128, 18] PSUM
  softmax without max-subtraction (|att| small); mask applied as a
  0/1 multiply after exp; normalization folded into the output scale
  out[b, D] = (sum_c diag(exp)_c^T @ regionN_c) * (1/S[b])
"""

import ml_dtypes
import numpy as np

T, N, B, D, A = 32, 36, 64, 512, 128
N_CORES = 8
T_LOC = T // N_CORES           # 4
ROWS = N * B                   # 2304
NCH = ROWS // 128              # 18
NDCH = D // 128                # 4
TB = T_LOC * B                 # 256
GROUPS = [(0, 512), (512, 512), (1024, 512), (1536, 512), (2048, 256)]

# packed bf16 consts blob column offsets
CB_FRAMET = 0                  # [J*256 + tb], 1024 cols
CB_WR = 1024                   # [J*128 + a], 512 cols
CB_WF = 1536                   # [J*128 + a], 512 cols
CB_WFULL = 2048                # 1 col
CB_DIAG = 2049                 # 64 cols
CB_I64 = 2113                  # 512 cols
CB_COLS = 2625
# packed f32 consts blob
CF_BATT = 0                    # 1 col
CF_DIAG = 1                    # 64 cols
CF_COLS = 65

_NC_CACHE = {}


def _build_nc(iters=1):
    import concourse.bacc as bacc
    from concourse import mybir
    from concourse.tile import TileContext

    f32 = mybir.dt.float32
    bf16 = mybir.dt.bfloat16
    f8 = mybir.dt.float8e4
    AF = mybir.ActivationFunctionType

    nc = bacc.Bacc(
        "TRN2", target_bir_lowering=False, debug=False, num_devices=N_CORES
    )
    # big per-t inputs, pre-laid-out on host for descriptor-clean DMAs
    rt = nc.dram_tensor("rt", [T_LOC, 128, NDCH, ROWS], f8, kind="ExternalInput")
    rn = nc.dram_tensor("rn", [T_LOC, 128, NCH, 512], bf16, kind="ExternalInput")
    keep = nc.dram_tensor("keep", [128, T_LOC, NCH], f32, kind="ExternalInput")
    cb = nc.dram_tensor("cb", [128, CB_COLS], bf16, kind="ExternalInput")
    cf = nc.dram_tensor("cf", [128, CF_COLS], f32, kind="ExternalInput")
    out = nc.dram_tensor("out", [T_LOC, B, D], f32, kind="ExternalOutput")

    with TileContext(nc) as tc:
        with (
            tc.tile_pool(name="consts", bufs=1) as consts,
            tc.tile_pool(name="rtp", bufs=4) as rtp,
            tc.tile_pool(name="rnp", bufs=4) as rnp,
            tc.tile_pool(name="thp", bufs=4) as thp,
            tc.tile_pool(name="smallp", bufs=4) as smallp,
            tc.tile_pool(name="diagp", bufs=3) as diagp,
            tc.tile_pool(name="outp", bufs=2) as outp,
            tc.tile_pool(name="pph", bufs=1, space="PSUM") as pph,
            tc.tile_pool(name="psmall", bufs=2, space="PSUM") as psmall,
            tc.tile_pool(name="po", bufs=1, space="PSUM") as po,
        ):
            # ---- packed constants: 2 DMAs, first in the SP ring ----
            cb_sb = consts.tile([128, CB_COLS], bf16)
            nc.sync.dma_start(out=cb_sb, in_=cb.ap())
            cf_sb = consts.tile([128, CF_COLS], f32)
            nc.sync.dma_start(out=cf_sb, in_=cf.ap())

            def frameT_s(J, h):
                o = CB_FRAMET + J * 256 + h * 128
                return cb_sb[:, o : o + 128]

            def wr_s(J):
                return cb_sb[:, CB_WR + J * 128 : CB_WR + (J + 1) * 128]

            def wf_s(J):
                return cb_sb[:, CB_WF + J * 128 : CB_WF + (J + 1) * 128]

            wfull_s = cb_sb[:, CB_WFULL : CB_WFULL + 1]
            diag01b_s = cb_sb[:, CB_DIAG : CB_DIAG + 64]
            battc_s = cf_sb[:, CF_BATT : CF_BATT + 1]
            diag01_s = cf_sb[:, CF_DIAG : CF_DIAG + 64]

            # ---- preamble: fproj[(t b), A] = frame @ Wf  (b_att via ACT) ----
            fproj_sb = consts.tile([128, 2, A], bf16)
            for h in range(2):
                pf = psmall.tile([128, A], f32, tag="s", name=f"pf{h}")
                for J in range(NDCH):
                    nc.tensor.matmul(
                        pf,
                        lhsT=frameT_s(J, h),
                        rhs=wf_s(J),
                        start=(J == 0),
                        stop=(J == NDCH - 1),
                    )
                nc.scalar.copy(out=fproj_sb[:, h, :], in_=pf)

            # ---- per-timestep body ----
            # two passes: rt loads + phase1/softmax for all t first (the
            # long dependent chain), then the rn-gated phase-2 streams, so
            # only the last few po matmuls trail the final rn bytes.
            def body(_iv=None):
                srts, srns, ths, expms, rss = [], [], [], [], []
                for t in range(T_LOC):
                    srt = rtp.tile([128, NDCH, ROWS], f8, tag="srt", name=f"rt{t}")
                    nc.sync.dma_start(out=srt, in_=rt.ap()[t])
                    srts.append(srt)
                for t in range(T_LOC):
                    srn = rnp.tile([128, NCH, 512], bf16, tag="srn", name=f"rn{t}")
                    # the last timestep's tail trails the final bytes: split
                    # it finer so fewer po matmuls wait on the last piece
                    splits = (
                        (0, 6), (6, 12), (12, 18)
                    ) if t < T_LOC - 1 else ((0, 6), (6, 12), (12, 15), (15, 18))
                    for lo, hi in splits:
                        nc.sync.dma_start(
                            out=srn[:, lo:hi, :], in_=rn.ap()[t, :, lo:hi, :]
                        )
                    srns.append(srn)
                skeep_all = smallp.tile(
                    [128, T_LOC, NCH], f32, tag="skeep", name="kp"
                )
                nc.scalar.dma_start(out=skeep_all, in_=keep.ap())

                # pass 1: phase 1 + att + masked-softmax stats per t
                for t in range(T_LOC):
                    srt = srts[t]
                    skeep = skeep_all[:, t, :]
                    rlo = (t % 2) * 64
                    th = thp.tile([128, ROWS], bf16, tag="th", name=f"th{t}")
                    ths.append(th)
                    ph_tiles = []
                    for g, (c0, cw) in enumerate(GROUPS):
                        ph_g = pph.tile([128, 512], f32, tag=f"ph{g}", name=f"ph{t}_{g}")
                        ph_tiles.append(ph_g)
                        nc.tensor.matmul(
                            ph_g[:, :cw],
                            lhsT=fproj_sb[rlo : rlo + 64, t // 2, :],
                            rhs=cb_sb[rlo : rlo + 64, CB_I64 : CB_I64 + cw],
                            start=True,
                            stop=False,
                        )
                    for J in range(NDCH):
                        for g, (c0, cw) in enumerate(GROUPS):
                            nc.tensor.matmul(
                                ph_tiles[g][:, :cw],
                                lhsT=wr_s(J),
                                rhs=srt[:, J, c0 : c0 + cw],
                                start=False,
                                stop=(J == NDCH - 1),
                            )
                    for g, (c0, cw) in enumerate(GROUPS):
                        # tanh(x + b_att[a]) with per-partition bias
                        nc.scalar.activation(
                            out=th[:, c0 : c0 + cw],
                            in_=ph_tiles[g][:, :cw],
                            func=AF.Tanh,
                            bias=battc_s,
                            scale=1.0,
                        )

                    # att columns: patt[:, c] = th_c^T @ wfull
                    patt = psmall.tile([128, NCH], f32, tag="s", name=f"pa{t}")
                    for c in range(NCH):
                        nc.tensor.matmul(
                            patt[:, c : c + 1],
                            lhsT=th[:, c * 128 : (c + 1) * 128],
                            rhs=wfull_s,
                            start=True,
                            stop=True,
                        )

                    # masked softmax pieces (normalization deferred)
                    expr = smallp.tile([128, NCH], f32, tag="expr", name=f"ex{t}")
                    nc.scalar.activation(out=expr, in_=patt, func=AF.Exp)
                    expm = smallp.tile([128, NCH], f32, tag="expm", name=f"em{t}")
                    nc.vector.tensor_mul(expm, expr, skeep)
                    expms.append(expm)
                    sacc = smallp.tile([128, 1], f32, tag="sacc", name=f"sa{t}")
                    nc.vector.tensor_reduce(
                        out=sacc,
                        in_=expm,
                        axis=mybir.AxisListType.X,
                        op=mybir.AluOpType.add,
                    )
                    ps64 = psmall.tile([64, 1], f32, tag="s", name=f"ps{t}")
                    nc.tensor.matmul(
                        ps64, lhsT=diag01_s, rhs=sacc, start=True, stop=True
                    )
                    rs = smallp.tile([64, 1], f32, tag="rs", name=f"rs{t}")
                    nc.vector.reciprocal(out=rs, in_=ps64)
                    rss.append(rs)

                # pass 2: po[b, D] = sum_c diag(expm_c)^T @ rN_c per t
                for t in range(T_LOC):
                    expm = expms[t]
                    po_t = po.tile([64, 512], f32, tag="po", name=f"po{t}")
                    for c in range(NCH):
                        dg = diagp.tile([128, 64], bf16, tag="dg", name=f"dg{t}_{c}")
                        nc.vector.tensor_scalar_mul(
                            out=dg, in0=diag01b_s, scalar1=expm[:, c : c + 1]
                        )
                        nc.tensor.matmul(
                            po_t,
                            lhsT=dg,
                            rhs=srns[t][:, c, :],
                            start=(c == 0),
                            stop=(c == NCH - 1),
                        )
                    osb = outp.tile([64, 512], f32, tag="osb", name=f"ob{t}")
                    nc.vector.tensor_scalar_mul(out=osb, in0=po_t, scalar1=rss[t])
                    nc.scalar.dma_start(out=out.ap()[t], in_=osb)

            if iters == 1:
                body()
            else:
                # body far exceeds one 16 KiB IRAM block per engine; arm the
                # back-edge branch prefetcher to avoid a ~3-4us I$-miss stall
                # per iteration
                with tc.For_i(
                    0, iters, 1, hint_engines=tuple(mybir.ALL_ENGINES)
                ) as iv:
                    body(iv)

    nc.compile()
    return nc


def _get_nc(iters=1):
    key = iters
    if key not in _NC_CACHE:
        _NC_CACHE[key] = _build_nc(iters)
    return _NC_CACHE[key]


def _make_in_maps(region_feat, frame_feat, mask, W_att, b_att, W_full):
    bf = ml_dtypes.bfloat16
    f8 = ml_dtypes.float8_e4m3
    region_f = np.asarray(region_feat, np.float32)
    # natural bf16: [T, 128p, 18c, 512] with row = 128c + p
    rn_all = np.ascontiguousarray(
        region_f.astype(bf).reshape(T, NCH, 128, D).transpose(0, 2, 1, 3)
    )
    # transposed fp8: [T, 128p, 4J, 2304r] with d = 128J + p
    rt_all = np.ascontiguousarray(
        region_f.astype(f8).reshape(T, ROWS, NDCH, 128).transpose(0, 3, 2, 1)
    )
    # keep-mask per core: [128p, T_LOC, 18c] f32, keep = 1 - mask
    keep_all = np.ascontiguousarray(
        (~np.asarray(mask, bool)).astype(np.float32)
        .reshape(N_CORES, T_LOC, NCH, 128).transpose(0, 3, 1, 2)
    )

    W_att = np.asarray(W_att, np.float32)
    wr_p = W_att[:D].astype(bf).reshape(NDCH, 128, A).transpose(1, 0, 2)  # [128,J,A]
    wf_p = W_att[D:].astype(bf).reshape(NDCH, 128, A).transpose(1, 0, 2)

    diag01 = np.zeros((128, 64), np.float32)
    diag01[np.arange(128), np.arange(128) % 64] = 1.0
    i64b = np.arange(128)[:, None] % 64 == np.arange(512)[None, :] % 64

    cf_blob = np.zeros((128, CF_COLS), np.float32)
    cf_blob[:, CF_BATT] = np.asarray(b_att, np.float32)
    cf_blob[:, CF_DIAG : CF_DIAG + 64] = diag01

    frame_bf = np.asarray(frame_feat, np.float32).astype(bf)  # [T,B,D]
    in_maps = []
    for cidx in range(N_CORES):
        sl = slice(cidx * T_LOC, (cidx + 1) * T_LOC)
        # frameT: [128p, (J, tb)] with d = 128J + p, core-local (t b)
        fT = np.ascontiguousarray(
            frame_bf[sl].reshape(TB, NDCH, 128).transpose(2, 1, 0)
        )
        cb_blob = np.zeros((128, CB_COLS), bf)
        cb_blob[:, CB_FRAMET : CB_FRAMET + NDCH * TB] = fT.reshape(128, NDCH * TB)
        cb_blob[:, CB_WR : CB_WR + NDCH * A] = wr_p.reshape(128, NDCH * A)
        cb_blob[:, CB_WF : CB_WF + NDCH * A] = wf_p.reshape(128, NDCH * A)
        cb_blob[:, CB_WFULL] = np.asarray(W_full, np.float32).astype(bf)
        cb_blob[:, CB_DIAG : CB_DIAG + 64] = diag01.astype(bf)
        cb_blob[:, CB_I64 : CB_I64 + 512] = i64b.astype(bf)
        in_maps.append(
            {
                "rt": rt_all[sl],
                "rn": rn_all[sl],
                "keep": keep_all[cidx],
                "cb": cb_blob,
                "cf": cf_blob,
            }
        )
    return in_maps


def kernel(region_feat, frame_feat, mask, W_att, b_att, W_full, b_full=None):
    """Full-input entry point. b_full is accepted but unused: softmax is
    invariant to a constant shift of the logits."""
    from concourse.bass_utils import run_bass_kernel_spmd

    nc = _get_nc()
    in_maps = _make_in_maps(region_feat, frame_feat, mask, W_att, b_att, W_full)
    res = run_bass_kernel_spmd(nc, in_maps, core_ids=list(range(N_CORES)))
    return np.concatenate(
        [res.results[c]["out"] for c in range(N_CORES)], axis=0
    ).astype(np.float32)
